# revision 14
# baseline (speedup 1.0000x reference)
"""Chamfer distance loss kernel for Trainium2 (Bass/Tile), 8-core data parallel.

Problem: x, y [16, 2048, 3] fp32. Per batch b:
    P[i,j] = |x_i|^2 + |y_j|^2 - 2 x_i.y_j
    loss[b] = mean_j min_i P[i,j] + mean_i min_j P[i,j]

Strategy:
  - Shard batch dim: 2 batches per core across 8 cores, no cross-core comm.
  - P = -2*Q with Q[i,j] = x_i.y_j - 0.5|x_i|^2 - 0.5|y_j|^2, computed as ONE
    K=24 augmented matmul (bf16 triple-split for accuracy ~2^-27).
    min P == -2 * max Q.
  - Per m-tile the [128,2048] PSUM block is extracted once by an ACT copy
    to bf16 SBUF; dr (row max) runs as pair-batched bf16 max-trees on DVE
    (2x perf mode); dl (col max across the 16 m-tiles) runs as one paired
    [128,2N] running-max chain on DVE, merged at the end; the partition-axis
    max/add use GpSimd C-axis reduces (no PE transposes).
  - Means via ACT sum-accumulator (dl, with -2/N folded into the scale) and
    a DVE row-sum + GpSimd C-axis add (dr).
"""

import sys

if "/opt/trn_rl_repo" not in sys.path:
    sys.path.insert(0, "/opt/trn_rl_repo")

import numpy as np

B, N, D = 16, 2048, 3
NCORES = 8
BPC = B // NCORES  # batches per core
MT = N // 128  # 16 m-tiles

_CACHE = {}


def _build(matmul_dtype="bfloat16"):
    from contextlib import ExitStack

    import concourse.bass as bass
    import concourse.mybir as mybir
    import concourse.tile as tile
    from concourse import bacc

    f32 = mybir.dt.float32
    mm_dt = getattr(mybir.dt, matmul_dtype)
    AL = mybir.AluOpType
    X = mybir.AxisListType.X
    C = mybir.AxisListType.C

    nc = bacc.Bacc()
    x = nc.dram_tensor("x", [BPC, N, D], f32, kind="ExternalInput")
    y = nc.dram_tensor("y", [BPC, N, D], f32, kind="ExternalInput")
    o = nc.dram_tensor("o", [1, BPC], f32, kind="ExternalOutput")
    # DRAM bounce buffer for the aug assembly (SBUF->SBUF transposes can't
    # be expressed as one DMA; DRAM APs have no partition-dim constraint)
    scratch = nc.dram_tensor("augscratch", [BPC, 128, 2 * 24 * 16], mm_dt,
                             kind="Internal")

    with tile.TileContext(nc) as tc, ExitStack() as ctx:
        singles = ctx.enter_context(tc.tile_pool(name="singles", bufs=1))
        aug_pool = ctx.enter_context(tc.tile_pool(name="aug", bufs=2))
        nat_pool = ctx.enter_context(tc.tile_pool(name="nat", bufs=2))
        small_pool = ctx.enter_context(tc.tile_pool(name="small", bufs=3))
        cp_pool = ctx.enter_context(tc.tile_pool(name="cp", bufs=4))
        tree_pool = ctx.enter_context(tc.tile_pool(name="tree", bufs=2))
        run_pool = ctx.enter_context(tc.tile_pool(name="run", bufs=2))
        fin_pool = ctx.enter_context(tc.tile_pool(name="fin", bufs=2))
        mm_psum = ctx.enter_context(tc.tile_pool(name="mmps", bufs=2, space="PSUM"))

        ones16 = singles.tile([128, N // 128], f32)
        nc.vector.memset(ones16, 1.0)
        out_sb = singles.tile([1, BPC], f32)

        # bf16 triple-split augmented matmul, K=24 rows per operand:
        #   x ~ xh+xm+xl (bf16 levels ~1, 2^-9, 2^-18); kept products
        #   hh,hm,mh,hl,lh,mm give x.y to ~2^-27.  Norms -0.5|x|^2 are
        #   3-way split and paired with ones rows.
        # stage fields (unique, [128,16] each):
        #   3d+0,3d+1,3d+2 = h/m/l of component d; 9,10,11 = norm h/m/l;
        #   12 = ones
        LROWS = []
        RROWS = []
        for d in range(D):
            h, m_, l = 3 * d, 3 * d + 1, 3 * d + 2
            LROWS += [h, h, m_, h, l, m_]
            RROWS += [h, m_, h, l, h, m_]
        LROWS += [9, 10, 11, 12, 12, 12]
        RROWS += [12, 12, 12, 9, 10, 11]
        K = len(LROWS)  # 24

        # slot layout per side: slot r of lstage holds the field ROWS[r], so
        # the whole aug block moves as ONE transposing DMA per side instead
        # of 24 per-row DMAs (whose fixed DGE cost dominated the kernel).
        def slot_plan(rows):
            first = {}
            dups = []  # (src_slot, dst_slot)
            for r, f in enumerate(rows):
                if f == 12:
                    continue  # ones slots are memset directly
                if f in first:
                    dups.append((first[f], r))
                else:
                    first[f] = r
            return first, dups

        for b in range(BPC):
            Q = N // 128  # points per partition
            # both sides' slot data in one tile: slot gi*K+r = row r of side gi
            lstage = nat_pool.tile([128, 2 * K * Q], mm_dt, tag="stage")
            stv = lstage.rearrange("p (s q) -> p s q", s=2 * K)
            for gi, (side, src, rows) in enumerate(
                (("x", x, LROWS), ("y", y, RROWS))
            ):
                first, dups = slot_plan(rows)
                go = gi * K
                nat = nat_pool.tile([128, Q * D], f32, tag=f"nat{side}")
                nc.sync.dma_start(
                    out=nat, in_=src[b].rearrange("(p q) d -> p (q d)", p=128)
                )
                sq = nat_pool.tile([128, Q * D], f32, tag=f"sq{side}")
                nc.vector.tensor_mul(sq, nat, nat)
                nrm = small_pool.tile([128, Q], f32, tag=f"nrm{side}")
                nc.vector.tensor_reduce(
                    nrm, sq.rearrange("p (q d) -> p q d", d=D), axis=X,
                    op=AL.add,
                )
                nc.vector.tensor_scalar_mul(nrm, nrm, -0.5)

                def split3(val_f32, fidx, pool_tag):
                    # val_f32: [128, Q] fp32; writes bf16 h/m/l into
                    # the three lstage slots in fidx
                    t1 = nat_pool.tile([128, Q], f32, tag=f"{pool_tag}t1")
                    t2 = nat_pool.tile([128, Q], f32, tag=f"{pool_tag}t2")
                    nc.gpsimd.tensor_copy(fidx[0], val_f32)          # h
                    nc.vector.tensor_sub(t1, val_f32, fidx[0])
                    nc.gpsimd.tensor_copy(fidx[1], t1)               # m
                    nc.vector.tensor_sub(t2, t1, fidx[1])
                    nc.gpsimd.tensor_copy(fidx[2], t2)               # l

                natv = nat.rearrange("p (q d) -> p d q", d=D)
                for d in range(D):
                    split3(
                        natv[:, d, :],
                        [stv[:, go + first[3 * d + j], :] for j in range(3)],
                        f"c{side}",
                    )
                split3(
                    nrm, [stv[:, go + first[9 + j], :] for j in range(3)],
                    f"n{side}",
                )
                # ones slots are contiguous for both row patterns
                ones_slots = [r for r, f in enumerate(rows) if f == 12]
                o0 = ones_slots[0]
                assert ones_slots == list(range(o0, o0 + 3))
                nc.gpsimd.memset(
                    lstage[:, (go + o0) * Q : (go + o0 + 3) * Q], 1.0
                )
                for src_slot, dst_slot in dups:
                    nc.gpsimd.tensor_copy(
                        stv[:, go + dst_slot, :], stv[:, go + src_slot, :]
                    )

            # aug assembly via DRAM bounce, 2 DMAs total:
            #   aug[r, g*N + p*16+q] = lstage[p, (g*K+r)*16+q]
            aug = aug_pool.tile([K, 2 * N], mm_dt, tag="aug")
            nc.sync.dma_start(out=scratch[b], in_=lstage)
            for g in range(2):
                nc.sync.dma_start(
                    out=aug[0:K, g * N : (g + 1) * N].rearrange(
                        "r (p q) -> r p q", p=128
                    ),
                    in_=scratch[b][:, g * K * Q : (g + 1) * K * Q].rearrange(
                        "p (r q) -> r p q", r=K
                    ),
                )
            lhsT = aug[:, 0:N]
            rhs = aug[:, N : 2 * N]

            # ---- main loop, m-tile PAIRS ----
            # Extraction is all-ACT (copy PSUM -> bf16 SBUF). dr runs as a
            # pair-batched bf16 max-tree on DVE (2x mode). dl runs as one
            # paired running-max chain [128, 2N] holding even tiles in the
            # left half, odd in the right, merged at the end.
            drcol = fin_pool.tile([128, MT], f32, tag="drcol")
            run2 = run_pool.tile([128, 2 * N], mm_dt, tag="run2")
            for mp in range(MT // 2):
                cpp = run2 if mp == 0 else cp_pool.tile(
                    [128, 2 * N], mm_dt, tag="cp"
                )
                for half in range(2):
                    m = 2 * mp + half
                    psg = mm_psum.tile([128, N], f32, tag="mm")
                    for n in range(4):
                        nc.tensor.matmul(
                            psg[:, n * 512 : (n + 1) * 512],
                            lhsT=lhsT[:, m * 128 : (m + 1) * 128],
                            rhs=rhs[:, n * 512 : (n + 1) * 512],
                            start=True,
                            stop=True,
                        )
                    nc.scalar.copy(cpp[:, half * N : (half + 1) * N], psg)
                # dl running max over pairs (first: it carries the serial chain)
                if mp > 0:
                    nc.vector.tensor_max(run2, run2, cpp)
                # dr pair-tree: fold j within each tile, both tiles at once
                t1 = tree_pool.tile([128, 2048], mm_dt, tag="t1")
                t2 = tree_pool.tile([128, 1024], mm_dt, tag="t2")
                t3 = tree_pool.tile([128, 512], mm_dt, tag="t3")
                cv = cpp.rearrange("p (a h j) -> p a h j", a=2, h=2)
                nc.vector.tensor_max(
                    t1.rearrange("p (a j) -> p a j", a=2),
                    cv[:, :, 0, :], cv[:, :, 1, :],
                )
                tv1 = t1.rearrange("p (a h j) -> p a h j", a=2, h=2)
                nc.vector.tensor_max(
                    t2.rearrange("p (a j) -> p a j", a=2),
                    tv1[:, :, 0, :], tv1[:, :, 1, :],
                )
                tv2 = t2.rearrange("p (a h j) -> p a h j", a=2, h=2)
                nc.vector.tensor_max(
                    t3.rearrange("p (a j) -> p a j", a=2),
                    tv2[:, :, 0, :], tv2[:, :, 1, :],
                )
                nc.vector.tensor_reduce(
                    drcol[:, 2 * mp : 2 * mp + 2],
                    t3.rearrange("p (a c) -> p a c", a=2),
                    axis=X, op=AL.max,
                )

            # ---- finals ----
            dlm = fin_pool.tile([128, N], mm_dt, tag="dlm")
            nc.vector.tensor_max(dlm, run2[:, 0:N], run2[:, N : 2 * N])
            dlrow = fin_pool.tile([1, N], f32, tag="dlrow")
            nc.gpsimd.tensor_reduce(dlrow, dlm, axis=C, op=AL.max)
            # dl mean with -2/N folded in: ACT accum = sum(dlrow * scale)
            junk = fin_pool.tile([1, N], mm_dt, tag="junk")
            dlsum = fin_pool.tile([1, 1], f32, tag="dlsum")
            nc.scalar.activation(
                junk, dlrow, mybir.ActivationFunctionType.Copy,
                scale=-2.0 / N, accum_out=dlsum,
            )
            # dr: sum over m-tiles then over partitions
            drs = fin_pool.tile([128, 1], f32, tag="drs")
            nc.vector.tensor_reduce(drs, drcol, axis=X, op=AL.add)
            drsum = fin_pool.tile([1, 1], f32, tag="drsum")
            nc.gpsimd.tensor_reduce(drsum, drs, axis=C, op=AL.add)
            tmp = fin_pool.tile([1, 1], f32, tag="tmp")
            nc.vector.tensor_scalar_mul(tmp, drsum, -2.0 / N)
            nc.vector.tensor_add(out_sb[0:1, b : b + 1], tmp, dlsum)

        nc.gpsimd.dma_start(out=o[0:1, 0:BPC], in_=out_sb)

    nc.compile()
    return nc


def _get_nc(matmul_dtype="bfloat16"):
    key = matmul_dtype
    if key not in _CACHE:
        _CACHE[key] = _build(matmul_dtype)
    return _CACHE[key]


def kernel(x: np.ndarray, y: np.ndarray) -> np.ndarray:
    from concourse.bass_utils import run_bass_kernel_spmd

    x = np.ascontiguousarray(np.asarray(x, dtype=np.float32))
    y = np.ascontiguousarray(np.asarray(y, dtype=np.float32))
    nc = _get_nc()
    in_maps = [
        {"x": x[c * BPC : (c + 1) * BPC], "y": y[c * BPC : (c + 1) * BPC]}
        for c in range(NCORES)
    ]
    res = run_bass_kernel_spmd(nc, in_maps, core_ids=list(range(NCORES)))
    return np.concatenate([r["o"].reshape(BPC) for r in res.results])


# revision 16
# speedup vs baseline: 3.4007x; 3.4007x over previous
"""Chamfer distance loss kernel for Trainium2 (Bass/Tile), 8-core data parallel.

Problem: x, y [16, 2048, 3] fp32. Per batch b:
    P[i,j] = |x_i|^2 + |y_j|^2 - 2 x_i.y_j
    loss[b] = mean_j min_i P[i,j] + mean_i min_j P[i,j]

Strategy:
  - Shard batch dim: 2 batches per core across 8 cores, no cross-core comm.
  - P = -2*Q with Q[i,j] = x_i.y_j - 0.5|x_i|^2 - 0.5|y_j|^2, computed as ONE
    K=24 augmented matmul (bf16 triple-split for accuracy ~2^-27).
    min P == -2 * max Q.
  - Per m-tile the [128,2048] PSUM block is extracted once by an ACT copy
    to bf16 SBUF; dr (row max) runs as pair-batched bf16 max-trees on DVE
    (2x perf mode); dl (col max across the 16 m-tiles) runs as one paired
    [128,2N] running-max chain on DVE, merged at the end; the partition-axis
    max/add use GpSimd C-axis reduces (no PE transposes).
  - Means via ACT sum-accumulator (dl, with -2/N folded into the scale) and
    a DVE row-sum + GpSimd C-axis add (dr).
"""

import sys

if "/opt/trn_rl_repo" not in sys.path:
    sys.path.insert(0, "/opt/trn_rl_repo")

import numpy as np

B, N, D = 16, 2048, 3
NCORES = 8
BPC = B // NCORES  # batches per core
MT = N // 128  # 16 m-tiles

_CACHE = {}


def _build(matmul_dtype="bfloat16"):
    from contextlib import ExitStack

    import concourse.bass as bass
    import concourse.mybir as mybir
    import concourse.tile as tile
    from concourse import bacc
    from concourse.masks import make_identity

    f32 = mybir.dt.float32
    mm_dt = getattr(mybir.dt, matmul_dtype)
    AL = mybir.AluOpType
    X = mybir.AxisListType.X
    C = mybir.AxisListType.C

    nc = bacc.Bacc()
    x = nc.dram_tensor("x", [BPC, N, D], f32, kind="ExternalInput")
    y = nc.dram_tensor("y", [BPC, N, D], f32, kind="ExternalInput")
    o = nc.dram_tensor("o", [1, BPC], f32, kind="ExternalOutput")
    # DRAM bounce buffer for the aug assembly (SBUF->SBUF transposes can't
    # be expressed as one DMA; DRAM APs have no partition-dim constraint)
    scratch = nc.dram_tensor("augscratch", [BPC, 128, 2 * 24 * 16], mm_dt,
                             kind="Internal")

    with tile.TileContext(nc) as tc, ExitStack() as ctx:
        singles = ctx.enter_context(tc.tile_pool(name="singles", bufs=1))
        aug_pool = ctx.enter_context(tc.tile_pool(name="aug", bufs=2))
        nat_pool = ctx.enter_context(tc.tile_pool(name="nat", bufs=2))
        small_pool = ctx.enter_context(tc.tile_pool(name="small", bufs=3))
        cp_pool = ctx.enter_context(tc.tile_pool(name="cp", bufs=4))
        tree_pool = ctx.enter_context(tc.tile_pool(name="tree", bufs=2))
        run_pool = ctx.enter_context(tc.tile_pool(name="run", bufs=2))
        fin_pool = ctx.enter_context(tc.tile_pool(name="fin", bufs=2))
        mm_psum = ctx.enter_context(tc.tile_pool(name="mmps", bufs=3, space="PSUM"))
        tp_psum = ctx.enter_context(tc.tile_pool(name="tpps", bufs=2, space="PSUM"))

        out_sb = singles.tile([1, BPC], f32)
        identity0 = singles.tile([128, 128], f32)
        make_identity(nc, identity0)
        identity = singles.tile([128, 128], mm_dt)
        nc.vector.tensor_copy(identity, identity0)

        # bf16 triple-split augmented matmul, K=24 rows per operand:
        #   x ~ xh+xm+xl (bf16 levels ~1, 2^-9, 2^-18); kept products
        #   hh,hm,mh,hl,lh,mm give x.y to ~2^-27.  Norms -0.5|x|^2 are
        #   3-way split and paired with ones rows.
        # stage fields (unique, [128,16] each):
        #   3d+0,3d+1,3d+2 = h/m/l of component d; 9,10,11 = norm h/m/l;
        #   12 = ones
        LROWS = []
        RROWS = []
        for d in range(D):
            h, m_, l = 3 * d, 3 * d + 1, 3 * d + 2
            LROWS += [h, h, m_, h, l, m_]
            RROWS += [h, m_, h, l, h, m_]
        LROWS += [9, 10, 11, 12, 12, 12]
        RROWS += [12, 12, 12, 9, 10, 11]
        K = len(LROWS)  # 24

        # slot layout per side: slot r of lstage holds the field ROWS[r], so
        # the whole aug block moves as ONE transposing DMA per side instead
        # of 24 per-row DMAs (whose fixed DGE cost dominated the kernel).
        def slot_plan(rows):
            first = {}
            dups = []  # (src_slot, dst_slot)
            for r, f in enumerate(rows):
                if f == 12:
                    continue  # ones slots are memset directly
                if f in first:
                    dups.append((first[f], r))
                else:
                    first[f] = r
            return first, dups

        for b in range(BPC):
            Q = N // 128  # points per partition
            # both sides' slot data in one tile: slot gi*K+r = row r of side gi
            lstage = nat_pool.tile([128, 2 * K * Q], mm_dt, tag="stage")
            stv = lstage.rearrange("p (s q) -> p s q", s=2 * K)
            for gi, (side, src, rows) in enumerate(
                (("x", x, LROWS), ("y", y, RROWS))
            ):
                first, dups = slot_plan(rows)
                go = gi * K
                nat = nat_pool.tile([128, Q * D], f32, tag=f"nat{side}")
                nc.sync.dma_start(
                    out=nat, in_=src[b].rearrange("(p q) d -> p (q d)", p=128)
                )
                sq = nat_pool.tile([128, Q * D], f32, tag=f"sq{side}")
                nc.vector.tensor_mul(sq, nat, nat)
                nrm = small_pool.tile([128, Q], f32, tag=f"nrm{side}")
                nc.vector.tensor_reduce(
                    nrm, sq.rearrange("p (q d) -> p q d", d=D), axis=X,
                    op=AL.add,
                )
                nc.vector.tensor_scalar_mul(nrm, nrm, -0.5)

                def split3(val_f32, fidx, pool_tag):
                    # val_f32: [128, Q] fp32; writes bf16 h/m/l into
                    # the three lstage slots in fidx
                    t1 = nat_pool.tile([128, Q], f32, tag=f"{pool_tag}t1")
                    t2 = nat_pool.tile([128, Q], f32, tag=f"{pool_tag}t2")
                    nc.gpsimd.tensor_copy(fidx[0], val_f32)          # h
                    nc.vector.tensor_sub(t1, val_f32, fidx[0])
                    nc.gpsimd.tensor_copy(fidx[1], t1)               # m
                    nc.vector.tensor_sub(t2, t1, fidx[1])
                    nc.gpsimd.tensor_copy(fidx[2], t2)               # l

                natv = nat.rearrange("p (q d) -> p d q", d=D)
                for d in range(D):
                    split3(
                        natv[:, d, :],
                        [stv[:, go + first[3 * d + j], :] for j in range(3)],
                        f"c{side}",
                    )
                split3(
                    nrm, [stv[:, go + first[9 + j], :] for j in range(3)],
                    f"n{side}",
                )
                # ones slots are contiguous for both row patterns
                ones_slots = [r for r, f in enumerate(rows) if f == 12]
                o0 = ones_slots[0]
                assert ones_slots == list(range(o0, o0 + 3))
                nc.gpsimd.memset(
                    lstage[:, (go + o0) * Q : (go + o0 + 3) * Q], 1.0
                )
                for src_slot, dst_slot in dups:
                    nc.gpsimd.tensor_copy(
                        stv[:, go + dst_slot, :], stv[:, go + src_slot, :]
                    )

            # aug assembly via DRAM bounce, 2 DMAs total:
            #   aug[r, g*N + p*16+q] = lstage[p, (g*K+r)*16+q]
            aug = aug_pool.tile([K, 2 * N], mm_dt, tag="aug")
            nc.sync.dma_start(out=scratch[b], in_=lstage)
            for g in range(2):
                nc.sync.dma_start(
                    out=aug[0:K, g * N : (g + 1) * N].rearrange(
                        "r (p q) -> r p q", p=128
                    ),
                    in_=scratch[b][:, g * K * Q : (g + 1) * K * Q].rearrange(
                        "p (r q) -> r p q", r=K
                    ),
                )
            lhsT = aug[:, 0:N]
            rhs = aug[:, N : 2 * N]

            # ---- main loop, m-tile PAIRS ----
            # Extraction is all-ACT (copy PSUM -> bf16 SBUF). dr runs as a
            # pair-batched bf16 max-tree on DVE (2x mode). dl runs as one
            # paired running-max chain [128, 2N] holding even tiles in the
            # left half, odd in the right, merged at the end.
            drcol = fin_pool.tile([128, MT], f32, tag="drcol")
            run2 = run_pool.tile([128, 2 * N], mm_dt, tag="run2")
            for mp in range(MT // 2):
                cpp = run2 if mp == 0 else cp_pool.tile(
                    [128, 2 * N], mm_dt, tag="cp"
                )
                for half in range(2):
                    m = 2 * mp + half
                    for jh in range(2):
                        psg = mm_psum.tile([128, N // 2], f32, tag="mm")
                        for n in range(2):
                            nc.tensor.matmul(
                                psg[:, n * 512 : (n + 1) * 512],
                                lhsT=lhsT[:, m * 128 : (m + 1) * 128],
                                rhs=rhs[
                                    :,
                                    (2 * jh + n) * 512 : (2 * jh + n + 1) * 512,
                                ],
                                start=True,
                                stop=True,
                            )
                        nc.scalar.copy(
                            cpp[
                                :,
                                half * N + jh * (N // 2) : half * N
                                + (jh + 1) * (N // 2),
                            ],
                            psg,
                        )
                # dl running max over pairs (first: it carries the serial chain)
                if mp > 0:
                    nc.vector.tensor_max(run2, run2, cpp)
                # dr pair-tree: fold j within each tile, both tiles at once
                t1 = tree_pool.tile([128, 2048], mm_dt, tag="t1")
                t2 = tree_pool.tile([128, 1024], mm_dt, tag="t2")
                t3 = tree_pool.tile([128, 512], mm_dt, tag="t3")
                cv = cpp.rearrange("p (a h j) -> p a h j", a=2, h=2)
                nc.vector.tensor_max(
                    t1.rearrange("p (a j) -> p a j", a=2),
                    cv[:, :, 0, :], cv[:, :, 1, :],
                )
                tv1 = t1.rearrange("p (a h j) -> p a h j", a=2, h=2)
                nc.vector.tensor_max(
                    t2.rearrange("p (a j) -> p a j", a=2),
                    tv1[:, :, 0, :], tv1[:, :, 1, :],
                )
                tv2 = t2.rearrange("p (a h j) -> p a h j", a=2, h=2)
                nc.vector.tensor_max(
                    t3.rearrange("p (a j) -> p a j", a=2),
                    tv2[:, :, 0, :], tv2[:, :, 1, :],
                )
                nc.vector.tensor_reduce(
                    drcol[:, 2 * mp : 2 * mp + 2],
                    t3.rearrange("p (a c) -> p a c", a=2),
                    axis=X, op=AL.max,
                )

            # ---- finals ----
            # dl partition-axis max via PE transposes (bf16) + DVE reduces;
            # the big GpSimd C-axis reduce measured ~160us on HW - unusable.
            dlm = fin_pool.tile([128, N], mm_dt, tag="dlm")
            nc.vector.tensor_max(dlm, run2[:, 0:N], run2[:, N : 2 * N])
            dlvals = fin_pool.tile([128, MT], f32, tag="dlvals")
            for th in range(2):
                tp = tp_psum.tile([128, 1024], mm_dt, tag="tp")
                for c in range(8):
                    blk = 8 * th + c
                    nc.tensor.transpose(
                        tp[:, c * 128 : (c + 1) * 128],
                        dlm[:, blk * 128 : (blk + 1) * 128],
                        identity,
                    )
                nc.vector.tensor_reduce(
                    dlvals[:, 8 * th : 8 * th + 8],
                    tp.rearrange("p (c j) -> p c j", c=8),
                    axis=X, op=AL.max,
                )
            # per-partition sums of dr and dl, then one tiny GpSimd C-add
            drs = fin_pool.tile([128, 1], f32, tag="drs")
            nc.vector.tensor_reduce(drs, drcol, axis=X, op=AL.add)
            dls = fin_pool.tile([128, 1], f32, tag="dls")
            nc.vector.tensor_reduce(dls, dlvals, axis=X, op=AL.add)
            tot = fin_pool.tile([128, 1], f32, tag="tot")
            nc.vector.tensor_add(tot, drs, dls)
            totsum = fin_pool.tile([1, 1], f32, tag="totsum")
            nc.gpsimd.tensor_reduce(totsum, tot, axis=C, op=AL.add)
            nc.vector.tensor_scalar_mul(
                out_sb[0:1, b : b + 1], totsum, -2.0 / N
            )

        nc.sync.dma_start(out=o[0:1, 0:BPC], in_=out_sb)

    nc.compile()
    return nc


def _get_nc(matmul_dtype="bfloat16"):
    key = matmul_dtype
    if key not in _CACHE:
        _CACHE[key] = _build(matmul_dtype)
    return _CACHE[key]


def kernel(x: np.ndarray, y: np.ndarray) -> np.ndarray:
    from concourse.bass_utils import run_bass_kernel_spmd

    x = np.ascontiguousarray(np.asarray(x, dtype=np.float32))
    y = np.ascontiguousarray(np.asarray(y, dtype=np.float32))
    nc = _get_nc()
    in_maps = [
        {"x": x[c * BPC : (c + 1) * BPC], "y": y[c * BPC : (c + 1) * BPC]}
        for c in range(NCORES)
    ]
    res = run_bass_kernel_spmd(nc, in_maps, core_ids=list(range(NCORES)))
    return np.concatenate([r["o"].reshape(BPC) for r in res.results])


# revision 17
# speedup vs baseline: 3.6658x; 1.0780x over previous
"""Chamfer distance loss kernel for Trainium2 (Bass/Tile), 8-core data parallel.

Problem: x, y [16, 2048, 3] fp32. Per batch b:
    P[i,j] = |x_i|^2 + |y_j|^2 - 2 x_i.y_j
    loss[b] = mean_j min_i P[i,j] + mean_i min_j P[i,j]

Strategy:
  - Shard batch dim: 2 batches per core across 8 cores, no cross-core comm.
  - P = -2*Q with Q[i,j] = x_i.y_j - 0.5|x_i|^2 - 0.5|y_j|^2, computed as ONE
    K=24 augmented matmul (bf16 triple-split for accuracy ~2^-27).
    min P == -2 * max Q.
  - Per m-tile the [128,2048] PSUM block is extracted once by an ACT copy
    to bf16 SBUF; dr (row max) runs as pair-batched bf16 max-trees on DVE
    (2x perf mode); dl (col max across the 16 m-tiles) runs as one paired
    [128,2N] running-max chain on DVE, merged at the end; the partition-axis
    max/add use GpSimd C-axis reduces (no PE transposes).
  - Means via ACT sum-accumulator (dl, with -2/N folded into the scale) and
    a DVE row-sum + GpSimd C-axis add (dr).
"""

import sys

if "/opt/trn_rl_repo" not in sys.path:
    sys.path.insert(0, "/opt/trn_rl_repo")

import numpy as np

B, N, D = 16, 2048, 3
NCORES = 8
BPC = B // NCORES  # batches per core
MT = N // 128  # 16 m-tiles

_CACHE = {}


def _build(matmul_dtype="bfloat16"):
    from contextlib import ExitStack

    import concourse.bass as bass
    import concourse.mybir as mybir
    import concourse.tile as tile
    from concourse import bacc
    from concourse.masks import make_identity

    f32 = mybir.dt.float32
    mm_dt = getattr(mybir.dt, matmul_dtype)
    AL = mybir.AluOpType
    X = mybir.AxisListType.X
    C = mybir.AxisListType.C

    nc = bacc.Bacc()
    x = nc.dram_tensor("x", [BPC, N, D], f32, kind="ExternalInput")
    y = nc.dram_tensor("y", [BPC, N, D], f32, kind="ExternalInput")
    o = nc.dram_tensor("o", [1, BPC], f32, kind="ExternalOutput")
    # DRAM bounce buffer for the aug assembly (SBUF->SBUF transposes can't
    # be expressed as one DMA; DRAM APs have no partition-dim constraint)
    scratch = nc.dram_tensor("augscratch", [BPC, 128, 2 * 24 * 16], mm_dt,
                             kind="Internal")

    with tile.TileContext(nc) as tc, ExitStack() as ctx:
        singles = ctx.enter_context(tc.tile_pool(name="singles", bufs=1))
        aug_pool = ctx.enter_context(tc.tile_pool(name="aug", bufs=2))
        nat_pool = ctx.enter_context(tc.tile_pool(name="nat", bufs=2))
        small_pool = ctx.enter_context(tc.tile_pool(name="small", bufs=3))
        cp_pool = ctx.enter_context(tc.tile_pool(name="cp", bufs=4))
        tree_pool = ctx.enter_context(tc.tile_pool(name="tree", bufs=2))
        run_pool = ctx.enter_context(tc.tile_pool(name="run", bufs=2))
        fin_pool = ctx.enter_context(tc.tile_pool(name="fin", bufs=2))
        mm_psum = ctx.enter_context(tc.tile_pool(name="mmps", bufs=3, space="PSUM"))
        tp_psum = ctx.enter_context(tc.tile_pool(name="tpps", bufs=2, space="PSUM"))

        out_sb = singles.tile([1, BPC], f32)
        identity0 = singles.tile([128, 128], f32)
        make_identity(nc, identity0)
        identity = singles.tile([128, 128], mm_dt)
        nc.vector.tensor_copy(identity, identity0)

        # bf16 triple-split augmented matmul, K=24 rows per operand:
        #   x ~ xh+xm+xl (bf16 levels ~1, 2^-9, 2^-18); kept products
        #   hh,hm,mh,hl,lh,mm give x.y to ~2^-27.  Norms -0.5|x|^2 are
        #   3-way split and paired with ones rows.
        # stage fields (unique, [128,16] each):
        #   3d+0,3d+1,3d+2 = h/m/l of component d; 9,10,11 = norm h/m/l;
        #   12 = ones
        LROWS = []
        RROWS = []
        for d in range(D):
            h, m_, l = 3 * d, 3 * d + 1, 3 * d + 2
            LROWS += [h, h, m_, h, l, m_]
            RROWS += [h, m_, h, l, h, m_]
        LROWS += [9, 10, 11, 12, 12, 12]
        RROWS += [12, 12, 12, 9, 10, 11]
        K = len(LROWS)  # 24

        # slot layout per side: slot r of lstage holds the field ROWS[r], so
        # the whole aug block moves as ONE transposing DMA per side instead
        # of 24 per-row DMAs (whose fixed DGE cost dominated the kernel).
        def slot_plan(rows):
            first = {}
            dups = []  # (src_slot, dst_slot)
            for r, f in enumerate(rows):
                if f == 12:
                    continue  # ones slots are memset directly
                if f in first:
                    dups.append((first[f], r))
                else:
                    first[f] = r
            return first, dups

        for b in range(BPC):
            Q = N // 128  # points per partition
            # both sides' slot data in one tile: slot gi*K+r = row r of side gi
            lstage = nat_pool.tile([128, 2 * K * Q], mm_dt, tag="stage")
            stv = lstage.rearrange("p (s q) -> p s q", s=2 * K)
            for gi, (side, src, rows) in enumerate(
                (("x", x, LROWS), ("y", y, RROWS))
            ):
                first, dups = slot_plan(rows)
                go = gi * K
                nat = nat_pool.tile([128, Q * D], f32, tag=f"nat{side}")
                nc.sync.dma_start(
                    out=nat, in_=src[b].rearrange("(p q) d -> p (q d)", p=128)
                )
                sq = nat_pool.tile([128, Q * D], f32, tag=f"sq{side}")
                nc.vector.tensor_mul(sq, nat, nat)
                nrm = small_pool.tile([128, Q], f32, tag=f"nrm{side}")
                nc.vector.tensor_reduce(
                    nrm, sq.rearrange("p (q d) -> p q d", d=D), axis=X,
                    op=AL.add,
                )
                nc.vector.tensor_scalar_mul(nrm, nrm, -0.5)

                def split3(val_f32, fidx, pool_tag):
                    # val_f32: [128, Q] fp32; writes bf16 h/m/l into
                    # the three lstage slots in fidx
                    t1 = nat_pool.tile([128, Q], f32, tag=f"{pool_tag}t1")
                    t2 = nat_pool.tile([128, Q], f32, tag=f"{pool_tag}t2")
                    nc.vector.tensor_copy(fidx[0], val_f32)          # h
                    nc.vector.tensor_sub(t1, val_f32, fidx[0])
                    nc.vector.tensor_copy(fidx[1], t1)               # m
                    nc.vector.tensor_sub(t2, t1, fidx[1])
                    nc.vector.tensor_copy(fidx[2], t2)               # l

                natv = nat.rearrange("p (q d) -> p d q", d=D)
                for d in range(D):
                    split3(
                        natv[:, d, :],
                        [stv[:, go + first[3 * d + j], :] for j in range(3)],
                        f"c{side}",
                    )
                split3(
                    nrm, [stv[:, go + first[9 + j], :] for j in range(3)],
                    f"n{side}",
                )
                # ones slots are contiguous for both row patterns
                ones_slots = [r for r, f in enumerate(rows) if f == 12]
                o0 = ones_slots[0]
                assert ones_slots == list(range(o0, o0 + 3))
                nc.gpsimd.memset(
                    lstage[:, (go + o0) * Q : (go + o0 + 3) * Q], 1.0
                )
                for src_slot, dst_slot in dups:
                    nc.gpsimd.tensor_copy(
                        stv[:, go + dst_slot, :], stv[:, go + src_slot, :]
                    )

            # aug assembly via DRAM bounce, 2 DMAs total:
            #   aug[r, g*N + p*16+q] = lstage[p, (g*K+r)*16+q]
            aug = aug_pool.tile([K, 2 * N], mm_dt, tag="aug")
            nc.sync.dma_start(out=scratch[b], in_=lstage)
            for g in range(2):
                nc.sync.dma_start(
                    out=aug[0:K, g * N : (g + 1) * N].rearrange(
                        "r (p q) -> r p q", p=128
                    ),
                    in_=scratch[b][:, g * K * Q : (g + 1) * K * Q].rearrange(
                        "p (r q) -> r p q", r=K
                    ),
                )
            lhsT = aug[:, 0:N]
            rhs = aug[:, N : 2 * N]

            # ---- main loop, m-tile PAIRS ----
            # Extraction is all-ACT (copy PSUM -> bf16 SBUF). dr runs as a
            # pair-batched bf16 max-tree on DVE (2x mode). dl runs as one
            # paired running-max chain [128, 2N] holding even tiles in the
            # left half, odd in the right, merged at the end.
            drcol = fin_pool.tile([128, MT], f32, tag="drcol")
            run2 = run_pool.tile([128, 2 * N], mm_dt, tag="run2")
            for mp in range(MT // 2):
                cpp = run2 if mp == 0 else cp_pool.tile(
                    [128, 2 * N], mm_dt, tag="cp"
                )
                for half in range(2):
                    m = 2 * mp + half
                    for jh in range(2):
                        psg = mm_psum.tile([128, N // 2], f32, tag="mm")
                        for n in range(2):
                            nc.tensor.matmul(
                                psg[:, n * 512 : (n + 1) * 512],
                                lhsT=lhsT[:, m * 128 : (m + 1) * 128],
                                rhs=rhs[
                                    :,
                                    (2 * jh + n) * 512 : (2 * jh + n + 1) * 512,
                                ],
                                start=True,
                                stop=True,
                            )
                        nc.scalar.copy(
                            cpp[
                                :,
                                half * N + jh * (N // 2) : half * N
                                + (jh + 1) * (N // 2),
                            ],
                            psg,
                        )
                # dl running max over pairs (first: it carries the serial chain)
                if mp > 0:
                    nc.vector.tensor_max(run2, run2, cpp)
                # dr pair-tree: fold j within each tile, both tiles at once
                t1 = tree_pool.tile([128, 2048], mm_dt, tag="t1")
                t2 = tree_pool.tile([128, 1024], mm_dt, tag="t2")
                t3 = tree_pool.tile([128, 512], mm_dt, tag="t3")
                cv = cpp.rearrange("p (a h j) -> p a h j", a=2, h=2)
                nc.vector.tensor_max(
                    t1.rearrange("p (a j) -> p a j", a=2),
                    cv[:, :, 0, :], cv[:, :, 1, :],
                )
                tv1 = t1.rearrange("p (a h j) -> p a h j", a=2, h=2)
                nc.vector.tensor_max(
                    t2.rearrange("p (a j) -> p a j", a=2),
                    tv1[:, :, 0, :], tv1[:, :, 1, :],
                )
                tv2 = t2.rearrange("p (a h j) -> p a h j", a=2, h=2)
                nc.vector.tensor_max(
                    t3.rearrange("p (a j) -> p a j", a=2),
                    tv2[:, :, 0, :], tv2[:, :, 1, :],
                )
                nc.vector.tensor_reduce(
                    drcol[:, 2 * mp : 2 * mp + 2],
                    t3.rearrange("p (a c) -> p a c", a=2),
                    axis=X, op=AL.max,
                )

            # ---- finals ----
            # dl partition-axis max via PE transposes (bf16) + DVE reduces;
            # the big GpSimd C-axis reduce measured ~160us on HW - unusable.
            dlm = fin_pool.tile([128, N], mm_dt, tag="dlm")
            nc.vector.tensor_max(dlm, run2[:, 0:N], run2[:, N : 2 * N])
            dlvals = fin_pool.tile([128, MT], f32, tag="dlvals")
            for th in range(2):
                tp = tp_psum.tile([128, 1024], mm_dt, tag="tp")
                for c in range(8):
                    blk = 8 * th + c
                    nc.tensor.transpose(
                        tp[:, c * 128 : (c + 1) * 128],
                        dlm[:, blk * 128 : (blk + 1) * 128],
                        identity,
                    )
                nc.vector.tensor_reduce(
                    dlvals[:, 8 * th : 8 * th + 8],
                    tp.rearrange("p (c j) -> p c j", c=8),
                    axis=X, op=AL.max,
                )
            # per-partition sums of dr and dl, then one tiny GpSimd C-add
            drs = fin_pool.tile([128, 1], f32, tag="drs")
            nc.vector.tensor_reduce(drs, drcol, axis=X, op=AL.add)
            dls = fin_pool.tile([128, 1], f32, tag="dls")
            nc.vector.tensor_reduce(dls, dlvals, axis=X, op=AL.add)
            tot = fin_pool.tile([128, 1], f32, tag="tot")
            nc.vector.tensor_add(tot, drs, dls)
            totsum = fin_pool.tile([1, 1], f32, tag="totsum")
            nc.gpsimd.tensor_reduce(totsum, tot, axis=C, op=AL.add)
            nc.vector.tensor_scalar_mul(
                out_sb[0:1, b : b + 1], totsum, -2.0 / N
            )

        nc.sync.dma_start(out=o[0:1, 0:BPC], in_=out_sb)

    nc.compile()
    return nc


def _get_nc(matmul_dtype="bfloat16"):
    key = matmul_dtype
    if key not in _CACHE:
        _CACHE[key] = _build(matmul_dtype)
    return _CACHE[key]


def kernel(x: np.ndarray, y: np.ndarray) -> np.ndarray:
    from concourse.bass_utils import run_bass_kernel_spmd

    x = np.ascontiguousarray(np.asarray(x, dtype=np.float32))
    y = np.ascontiguousarray(np.asarray(y, dtype=np.float32))
    nc = _get_nc()
    in_maps = [
        {"x": x[c * BPC : (c + 1) * BPC], "y": y[c * BPC : (c + 1) * BPC]}
        for c in range(NCORES)
    ]
    res = run_bass_kernel_spmd(nc, in_maps, core_ids=list(range(NCORES)))
    return np.concatenate([r["o"].reshape(BPC) for r in res.results])


# revision 19
# speedup vs baseline: 3.6983x; 1.0089x over previous
"""Chamfer distance loss kernel for Trainium2 (Bass/Tile), 8-core data parallel.

Problem: x, y [16, 2048, 3] fp32. Per batch b:
    P[i,j] = |x_i|^2 + |y_j|^2 - 2 x_i.y_j
    loss[b] = mean_j min_i P[i,j] + mean_i min_j P[i,j]

Strategy:
  - Shard batch dim: 2 batches per core across 8 cores, no cross-core comm.
  - P = -2*Q with Q[i,j] = x_i.y_j - 0.5|x_i|^2 - 0.5|y_j|^2, computed as ONE
    K=24 augmented matmul (bf16 triple-split for accuracy ~2^-27).
    min P == -2 * max Q.
  - Per m-tile the [128,2048] PSUM block is extracted once by an ACT copy
    to bf16 SBUF; dr (row max) runs as pair-batched bf16 max-trees on DVE
    (2x perf mode); dl (col max across the 16 m-tiles) runs as one paired
    [128,2N] running-max chain on DVE, merged at the end; the partition-axis
    max/add use GpSimd C-axis reduces (no PE transposes).
  - Means via ACT sum-accumulator (dl, with -2/N folded into the scale) and
    a DVE row-sum + GpSimd C-axis add (dr).
"""

import sys

if "/opt/trn_rl_repo" not in sys.path:
    sys.path.insert(0, "/opt/trn_rl_repo")

import numpy as np

B, N, D = 16, 2048, 3
NCORES = 8
BPC = B // NCORES  # batches per core
MT = N // 128  # 16 m-tiles

_CACHE = {}


def _build(matmul_dtype="bfloat16"):
    from contextlib import ExitStack

    import concourse.bass as bass
    import concourse.mybir as mybir
    import concourse.tile as tile
    from concourse import bacc
    from concourse.masks import make_identity

    f32 = mybir.dt.float32
    mm_dt = getattr(mybir.dt, matmul_dtype)
    AL = mybir.AluOpType
    X = mybir.AxisListType.X
    C = mybir.AxisListType.C

    nc = bacc.Bacc()
    x = nc.dram_tensor("x", [BPC, N, D], f32, kind="ExternalInput")
    y = nc.dram_tensor("y", [BPC, N, D], f32, kind="ExternalInput")
    o = nc.dram_tensor("o", [1, BPC], f32, kind="ExternalOutput")
    # DRAM bounce buffer for the aug assembly (SBUF->SBUF transposes can't
    # be expressed as one DMA; DRAM APs have no partition-dim constraint)
    scratch = nc.dram_tensor("augscratch", [BPC, 128, 2 * 24 * 16], mm_dt,
                             kind="Internal")

    with tile.TileContext(nc) as tc, ExitStack() as ctx:
        singles = ctx.enter_context(tc.tile_pool(name="singles", bufs=1))
        aug_pool = ctx.enter_context(tc.tile_pool(name="aug", bufs=2))
        nat_pool = ctx.enter_context(tc.tile_pool(name="nat", bufs=2))
        small_pool = ctx.enter_context(tc.tile_pool(name="small", bufs=3))
        cp_pool = ctx.enter_context(tc.tile_pool(name="cp", bufs=4))
        tree_pool = ctx.enter_context(tc.tile_pool(name="tree", bufs=2))
        run_pool = ctx.enter_context(tc.tile_pool(name="run", bufs=2))
        fin_pool = ctx.enter_context(tc.tile_pool(name="fin", bufs=2))
        mm_psum = ctx.enter_context(tc.tile_pool(name="mmps", bufs=3, space="PSUM"))
        tp_psum = ctx.enter_context(tc.tile_pool(name="tpps", bufs=2, space="PSUM"))

        out_sb = singles.tile([1, BPC], f32)
        identity0 = singles.tile([128, 128], f32)
        make_identity(nc, identity0)
        identity = singles.tile([128, 128], mm_dt)
        nc.vector.tensor_copy(identity, identity0)

        # bf16 triple-split augmented matmul, K=24 rows per operand:
        #   x ~ xh+xm+xl (bf16 levels ~1, 2^-9, 2^-18); kept products
        #   hh,hm,mh,hl,lh,mm give x.y to ~2^-27.  Norms -0.5|x|^2 are
        #   3-way split and paired with ones rows.
        # stage fields (unique, [128,16] each):
        #   3d+0,3d+1,3d+2 = h/m/l of component d; 9,10,11 = norm h/m/l;
        #   12 = ones
        LROWS = []
        RROWS = []
        for d in range(D):
            h, m_, l = 3 * d, 3 * d + 1, 3 * d + 2
            LROWS += [h, h, m_, h, l, m_]
            RROWS += [h, m_, h, l, h, m_]
        LROWS += [9, 10, 11, 12, 12, 12]
        RROWS += [12, 12, 12, 9, 10, 11]
        K = len(LROWS)  # 24

        # slot layout per side: slot r of lstage holds the field ROWS[r], so
        # the whole aug block moves as ONE transposing DMA per side instead
        # of 24 per-row DMAs (whose fixed DGE cost dominated the kernel).
        def slot_plan(rows):
            first = {}
            dups = []  # (src_slot, dst_slot)
            for r, f in enumerate(rows):
                if f == 12:
                    continue  # ones slots are memset directly
                if f in first:
                    dups.append((first[f], r))
                else:
                    first[f] = r
            return first, dups

        for b in range(BPC):
            Q = N // 128  # points per partition
            # both sides' slot data in one tile: slot gi*K+r = row r of side gi
            lstage = nat_pool.tile([128, 2 * K * Q], mm_dt, tag="stage")
            aug = aug_pool.tile([K, 2 * N], mm_dt, tag="aug")
            stv = lstage.rearrange("p (s q) -> p s q", s=2 * K)
            for gi, (side, src, rows) in enumerate(
                (("x", x, LROWS), ("y", y, RROWS))
            ):
                first, dups = slot_plan(rows)
                go = gi * K
                dma_eng = nc.sync if gi == 0 else nc.scalar
                nat = nat_pool.tile([128, Q * D], f32, tag=f"nat{side}")
                dma_eng.dma_start(
                    out=nat, in_=src[b].rearrange("(p q) d -> p (q d)", p=128)
                )
                sq = nat_pool.tile([128, Q * D], f32, tag=f"sq{side}")
                nc.vector.tensor_mul(sq, nat, nat)
                nrm = small_pool.tile([128, Q], f32, tag=f"nrm{side}")
                nc.vector.tensor_reduce(
                    nrm, sq.rearrange("p (q d) -> p q d", d=D), axis=X,
                    op=AL.add,
                )
                nc.vector.tensor_scalar_mul(nrm, nrm, -0.5)

                def split3(val_f32, fidx, pool_tag):
                    # val_f32: [128, Q] fp32; writes bf16 h/m/l into
                    # the three lstage slots in fidx
                    t1 = nat_pool.tile([128, Q], f32, tag=f"{pool_tag}t1")
                    t2 = nat_pool.tile([128, Q], f32, tag=f"{pool_tag}t2")
                    nc.vector.tensor_copy(fidx[0], val_f32)          # h
                    nc.vector.tensor_sub(t1, val_f32, fidx[0])
                    nc.vector.tensor_copy(fidx[1], t1)               # m
                    nc.vector.tensor_sub(t2, t1, fidx[1])
                    nc.vector.tensor_copy(fidx[2], t2)               # l

                natv = nat.rearrange("p (q d) -> p d q", d=D)
                for d in range(D):
                    split3(
                        natv[:, d, :],
                        [stv[:, go + first[3 * d + j], :] for j in range(3)],
                        f"c{side}",
                    )
                split3(
                    nrm, [stv[:, go + first[9 + j], :] for j in range(3)],
                    f"n{side}",
                )
                # ones slots are contiguous for both row patterns
                ones_slots = [r for r, f in enumerate(rows) if f == 12]
                o0 = ones_slots[0]
                assert ones_slots == list(range(o0, o0 + 3))
                nc.gpsimd.memset(
                    lstage[:, (go + o0) * Q : (go + o0 + 3) * Q], 1.0
                )
                for src_slot, dst_slot in dups:
                    nc.gpsimd.tensor_copy(
                        stv[:, go + dst_slot, :], stv[:, go + src_slot, :]
                    )
                # aug assembly via DRAM bounce, per side on its own queue:
                #   aug[r, g*N + p*16+q] = lstage[p, (g*K+r)*16+q]
                g = gi
                dma_eng.dma_start(
                    out=scratch[b][:, g * K * Q : (g + 1) * K * Q],
                    in_=lstage[:, g * K * Q : (g + 1) * K * Q],
                )
                dma_eng.dma_start(
                    out=aug[0:K, g * N : (g + 1) * N].rearrange(
                        "r (p q) -> r p q", p=128
                    ),
                    in_=scratch[b][:, g * K * Q : (g + 1) * K * Q].rearrange(
                        "p (r q) -> r p q", r=K
                    ),
                )

            lhsT = aug[:, 0:N]
            rhs = aug[:, N : 2 * N]

            # ---- main loop, m-tile PAIRS ----
            # Extraction is all-ACT (copy PSUM -> bf16 SBUF). dr runs as a
            # pair-batched bf16 max-tree on DVE (2x mode). dl runs as one
            # paired running-max chain [128, 2N] holding even tiles in the
            # left half, odd in the right, merged at the end.
            drcol = fin_pool.tile([128, MT], f32, tag="drcol")
            run4 = run_pool.tile([128, 4 * N], mm_dt, tag="run4")
            for mq in range(MT // 4):
                cpp = run4 if mq == 0 else cp_pool.tile(
                    [128, 4 * N], mm_dt, tag="cp"
                )
                for quar in range(4):
                    m = 4 * mq + quar
                    for jh in range(2):
                        psg = mm_psum.tile([128, N // 2], f32, tag="mm")
                        for n in range(2):
                            nc.tensor.matmul(
                                psg[:, n * 512 : (n + 1) * 512],
                                lhsT=lhsT[:, m * 128 : (m + 1) * 128],
                                rhs=rhs[
                                    :,
                                    (2 * jh + n) * 512 : (2 * jh + n + 1) * 512,
                                ],
                                start=True,
                                stop=True,
                            )
                        nc.scalar.copy(
                            cpp[
                                :,
                                quar * N + jh * (N // 2) : quar * N
                                + (jh + 1) * (N // 2),
                            ],
                            psg,
                        )
                # dl running max over quads (first: it carries the serial chain)
                if mq > 0:
                    nc.vector.tensor_max(run4, run4, cpp)
                # dr quad-tree: fold j within each tile, 4 tiles batched
                widths = [1024, 512, 256, 128]
                prev = cpp
                for w in widths:
                    nxt = tree_pool.tile([128, 4 * w], mm_dt, tag=f"t{w}")
                    pv = prev.rearrange("p (a h j) -> p a h j", a=4, h=2)
                    nc.vector.tensor_max(
                        nxt.rearrange("p (a j) -> p a j", a=4),
                        pv[:, :, 0, :], pv[:, :, 1, :],
                    )
                    prev = nxt
                nc.vector.tensor_reduce(
                    drcol[:, 4 * mq : 4 * mq + 4],
                    prev.rearrange("p (a c) -> p a c", a=4),
                    axis=X, op=AL.max,
                )

            # ---- finals ----
            # dl partition-axis max via PE transposes (bf16) + DVE reduces;
            # the big GpSimd C-axis reduce measured ~160us on HW - unusable.
            dlm2 = fin_pool.tile([128, 2 * N], mm_dt, tag="dlm2")
            nc.vector.tensor_max(
                dlm2, run4[:, 0 : 2 * N], run4[:, 2 * N : 4 * N]
            )
            dlm = fin_pool.tile([128, N], mm_dt, tag="dlm")
            nc.vector.tensor_max(dlm, dlm2[:, 0:N], dlm2[:, N : 2 * N])
            dlvals = fin_pool.tile([128, MT], f32, tag="dlvals")
            for th in range(2):
                tp = tp_psum.tile([128, 1024], mm_dt, tag="tp")
                for c in range(8):
                    blk = 8 * th + c
                    nc.tensor.transpose(
                        tp[:, c * 128 : (c + 1) * 128],
                        dlm[:, blk * 128 : (blk + 1) * 128],
                        identity,
                    )
                nc.vector.tensor_reduce(
                    dlvals[:, 8 * th : 8 * th + 8],
                    tp.rearrange("p (c j) -> p c j", c=8),
                    axis=X, op=AL.max,
                )
            # per-partition sums of dr and dl, then one tiny GpSimd C-add
            drs = fin_pool.tile([128, 1], f32, tag="drs")
            nc.vector.tensor_reduce(drs, drcol, axis=X, op=AL.add)
            dls = fin_pool.tile([128, 1], f32, tag="dls")
            nc.vector.tensor_reduce(dls, dlvals, axis=X, op=AL.add)
            tot = fin_pool.tile([128, 1], f32, tag="tot")
            nc.vector.tensor_add(tot, drs, dls)
            totsum = fin_pool.tile([1, 1], f32, tag="totsum")
            nc.gpsimd.tensor_reduce(totsum, tot, axis=C, op=AL.add)
            nc.vector.tensor_scalar_mul(
                out_sb[0:1, b : b + 1], totsum, -2.0 / N
            )

        nc.sync.dma_start(out=o[0:1, 0:BPC], in_=out_sb)

    nc.compile()
    return nc


def _get_nc(matmul_dtype="bfloat16"):
    key = matmul_dtype
    if key not in _CACHE:
        _CACHE[key] = _build(matmul_dtype)
    return _CACHE[key]


def kernel(x: np.ndarray, y: np.ndarray) -> np.ndarray:
    from concourse.bass_utils import run_bass_kernel_spmd

    x = np.ascontiguousarray(np.asarray(x, dtype=np.float32))
    y = np.ascontiguousarray(np.asarray(y, dtype=np.float32))
    nc = _get_nc()
    in_maps = [
        {"x": x[c * BPC : (c + 1) * BPC], "y": y[c * BPC : (c + 1) * BPC]}
        for c in range(NCORES)
    ]
    res = run_bass_kernel_spmd(nc, in_maps, core_ids=list(range(NCORES)))
    return np.concatenate([r["o"].reshape(BPC) for r in res.results])


# revision 21
# speedup vs baseline: 3.8299x; 1.0356x over previous
"""Chamfer distance loss kernel for Trainium2 (Bass/Tile), 8-core data parallel.

Problem: x, y [16, 2048, 3] fp32. Per batch b:
    P[i,j] = |x_i|^2 + |y_j|^2 - 2 x_i.y_j
    loss[b] = mean_j min_i P[i,j] + mean_i min_j P[i,j]

Strategy:
  - Shard batch dim: 2 batches per core across 8 cores, no cross-core comm.
  - P = -2*Q with Q[i,j] = x_i.y_j - 0.5|x_i|^2 - 0.5|y_j|^2, computed as ONE
    K=24 augmented matmul (bf16 triple-split for accuracy ~2^-27).
    min P == -2 * max Q.
  - Per m-tile the [128,2048] PSUM block is extracted once by an ACT copy
    to bf16 SBUF; dr (row max) runs as pair-batched bf16 max-trees on DVE
    (2x perf mode); dl (col max across the 16 m-tiles) runs as one paired
    [128,2N] running-max chain on DVE, merged at the end; the partition-axis
    max/add use GpSimd C-axis reduces (no PE transposes).
  - Means via ACT sum-accumulator (dl, with -2/N folded into the scale) and
    a DVE row-sum + GpSimd C-axis add (dr).
"""

import sys

if "/opt/trn_rl_repo" not in sys.path:
    sys.path.insert(0, "/opt/trn_rl_repo")

import numpy as np

B, N, D = 16, 2048, 3
NCORES = 8
BPC = B // NCORES  # batches per core
MT = N // 128  # 16 m-tiles

_CACHE = {}


def _build(matmul_dtype="float16"):
    from contextlib import ExitStack

    import concourse.bass as bass
    import concourse.mybir as mybir
    import concourse.tile as tile
    from concourse import bacc
    from concourse.masks import make_identity

    f32 = mybir.dt.float32
    mm_dt = getattr(mybir.dt, matmul_dtype)
    AL = mybir.AluOpType
    X = mybir.AxisListType.X
    C = mybir.AxisListType.C

    nc = bacc.Bacc()
    x = nc.dram_tensor("x", [BPC, N, D], f32, kind="ExternalInput")
    y = nc.dram_tensor("y", [BPC, N, D], f32, kind="ExternalInput")
    o = nc.dram_tensor("o", [1, BPC], f32, kind="ExternalOutput")
    # DRAM bounce buffer for the aug assembly (SBUF->SBUF transposes can't
    # be expressed as one DMA; DRAM APs have no partition-dim constraint)
    scratch = nc.dram_tensor("augscratch", [BPC, 128, 2 * 13 * 16], mm_dt,
                             kind="Internal")

    with tile.TileContext(nc) as tc, ExitStack() as ctx:
        singles = ctx.enter_context(tc.tile_pool(name="singles", bufs=1))
        aug_pool = ctx.enter_context(tc.tile_pool(name="aug", bufs=2))
        nat_pool = ctx.enter_context(tc.tile_pool(name="nat", bufs=2))
        small_pool = ctx.enter_context(tc.tile_pool(name="small", bufs=3))
        cp_pool = ctx.enter_context(tc.tile_pool(name="cp", bufs=4))
        tree_pool = ctx.enter_context(tc.tile_pool(name="tree", bufs=2))
        run_pool = ctx.enter_context(tc.tile_pool(name="run", bufs=2))
        fin_pool = ctx.enter_context(tc.tile_pool(name="fin", bufs=2))
        mm_psum = ctx.enter_context(tc.tile_pool(name="mmps", bufs=3, space="PSUM"))
        tp_psum = ctx.enter_context(tc.tile_pool(name="tpps", bufs=2, space="PSUM"))

        out_sb = singles.tile([1, BPC], f32)
        identity0 = singles.tile([128, 128], f32)
        make_identity(nc, identity0)
        identity = singles.tile([128, 128], mm_dt)
        nc.vector.tensor_copy(identity, identity0)

        # fp16 double-split augmented matmul, K=13 rows per operand:
        #   x ~ xh+xl (fp16 levels ~1, 2^-11); kept products hh,hl,lh give
        #   x.y to ~2^-21 (fp16 products are exact in fp32 accumulate).
        #   Norms -0.5|x|^2 are 2-way split and paired with ones rows.
        # stage fields (unique, [128,16] each):
        #   2d, 2d+1 = h/l of component d; 6,7 = norm h/l; 8 = ones
        LROWS = []
        RROWS = []
        for d in range(D):
            h, l = 2 * d, 2 * d + 1
            LROWS += [h, h, l]
            RROWS += [h, l, h]
        LROWS += [6, 7, 8, 8]
        RROWS += [8, 8, 6, 7]
        K = len(LROWS)  # 13

        # persistent per-batch lstage buffers; ones slots memset once at t=0
        lstages_all = []
        for bi in range(BPC):
            ls = singles.tile([128, 2 * K * 16], mm_dt, name=f"lst{bi}")
            lstages_all.append(ls)
            for g2, rows2 in enumerate((LROWS, RROWS)):
                oslots = [r for r, f in enumerate(rows2) if f == 8]
                o0 = oslots[0]
                assert oslots == list(range(o0, o0 + len(oslots)))
                nc.vector.memset(
                    ls[:, (g2 * K + o0) * 16 : (g2 * K + o0 + len(oslots)) * 16],
                    1.0,
                )

        # slot layout per side: slot r of lstage holds the field ROWS[r], so
        # the whole aug block moves as ONE transposing DMA per side instead
        # of 24 per-row DMAs (whose fixed DGE cost dominated the kernel).
        def slot_plan(rows):
            first = {}
            dups = []  # (src_slot, dst_slot)
            for r, f in enumerate(rows):
                if f == 8:
                    continue  # ones slots are memset directly
                if f in first:
                    dups.append((first[f], r))
                else:
                    first[f] = r
            return first, dups

        for b in range(BPC):
            Q = N // 128  # points per partition
            # both sides' slot data in one tile: slot gi*K+r = row r of side gi
            lstage = lstages_all[b]
            aug = aug_pool.tile([K, 2 * N], mm_dt, tag="aug")
            stv = lstage.rearrange("p (s q) -> p s q", s=2 * K)
            for gi, (side, src, rows) in enumerate(
                (("x", x, LROWS), ("y", y, RROWS))
            ):
                first, dups = slot_plan(rows)
                go = gi * K
                dma_eng = nc.sync if gi == 0 else nc.scalar
                nat = nat_pool.tile([128, Q * D], f32, tag=f"nat{side}")
                dma_eng.dma_start(
                    out=nat, in_=src[b].rearrange("(p q) d -> p (q d)", p=128)
                )
                sq = nat_pool.tile([128, Q * D], f32, tag=f"sq{side}")
                nc.vector.tensor_mul(sq, nat, nat)
                nrm = small_pool.tile([128, Q], f32, tag=f"nrm{side}")
                nc.vector.tensor_reduce(
                    nrm, sq.rearrange("p (q d) -> p q d", d=D), axis=X,
                    op=AL.add,
                )
                nc.vector.tensor_scalar_mul(nrm, nrm, -0.5)

                def split2(val_f32, fidx, pool_tag):
                    # val_f32: [128, Q] fp32; writes fp16 h/l into
                    # the two lstage slots in fidx
                    t1 = nat_pool.tile([128, Q], f32, tag=f"{pool_tag}t1")
                    nc.vector.tensor_copy(fidx[0], val_f32)          # h
                    nc.vector.tensor_sub(t1, val_f32, fidx[0])
                    nc.vector.tensor_copy(fidx[1], t1)               # l

                natv = nat.rearrange("p (q d) -> p d q", d=D)
                for d in range(D):
                    split2(
                        natv[:, d, :],
                        [stv[:, go + first[2 * d + j], :] for j in range(2)],
                        f"c{side}",
                    )
                split2(
                    nrm, [stv[:, go + first[6 + j], :] for j in range(2)],
                    f"n{side}",
                )
                # dup slots: batch 0 on DVE (in-engine, startup latency);
                # batch 1 on GpSimd (fully overlapped)
                dup_eng = nc.vector if b == 0 else nc.gpsimd
                for src_slot, dst_slot in dups:
                    dup_eng.tensor_copy(
                        stv[:, go + dst_slot, :], stv[:, go + src_slot, :]
                    )
                # aug assembly via DRAM bounce, per side on its own queue:
                #   aug[r, g*N + p*16+q] = lstage[p, (g*K+r)*16+q]
                g = gi
                dma_eng.dma_start(
                    out=scratch[b][:, g * K * Q : (g + 1) * K * Q],
                    in_=lstage[:, g * K * Q : (g + 1) * K * Q],
                )
                dma_eng.dma_start(
                    out=aug[0:K, g * N : (g + 1) * N].rearrange(
                        "r (p q) -> r p q", p=128
                    ),
                    in_=scratch[b][:, g * K * Q : (g + 1) * K * Q].rearrange(
                        "p (r q) -> r p q", r=K
                    ),
                )

            lhsT = aug[:, 0:N]
            rhs = aug[:, N : 2 * N]

            # ---- main loop, m-tile PAIRS ----
            # Extraction is all-ACT (copy PSUM -> bf16 SBUF). dr runs as a
            # pair-batched bf16 max-tree on DVE (2x mode). dl runs as one
            # paired running-max chain [128, 2N] holding even tiles in the
            # left half, odd in the right, merged at the end.
            drcol = fin_pool.tile([128, MT], f32, tag="drcol")
            run2 = run_pool.tile([128, 2 * N], mm_dt, tag="run2")
            for mq in range(MT // 4):
                cpp = cp_pool.tile([128, 4 * N], mm_dt, tag="cp")
                for quar in range(4):
                    m = 4 * mq + quar
                    for jh in range(2):
                        psg = mm_psum.tile([128, N // 2], f32, tag="mm")
                        for n in range(2):
                            nc.tensor.matmul(
                                psg[:, n * 512 : (n + 1) * 512],
                                lhsT=lhsT[:, m * 128 : (m + 1) * 128],
                                rhs=rhs[
                                    :,
                                    (2 * jh + n) * 512 : (2 * jh + n + 1) * 512,
                                ],
                                start=True,
                                stop=True,
                            )
                        nc.scalar.copy(
                            cpp[
                                :,
                                quar * N + jh * (N // 2) : quar * N
                                + (jh + 1) * (N // 2),
                            ],
                            psg,
                        )
                # dl running max, pair-width ops (first: serial chain priority)
                if mq == 0:
                    nc.vector.tensor_max(
                        run2, cpp[:, 0 : 2 * N], cpp[:, 2 * N : 4 * N]
                    )
                else:
                    nc.vector.tensor_max(run2, run2, cpp[:, 0 : 2 * N])
                    nc.vector.tensor_max(run2, run2, cpp[:, 2 * N : 4 * N])
                # dr quad-tree: fold j within each tile, 4 tiles batched
                widths = [1024, 512, 256, 128]
                prev = cpp
                for w in widths:
                    nxt = tree_pool.tile([128, 4 * w], mm_dt, tag=f"t{w}")
                    pv = prev.rearrange("p (a h j) -> p a h j", a=4, h=2)
                    nc.vector.tensor_max(
                        nxt.rearrange("p (a j) -> p a j", a=4),
                        pv[:, :, 0, :], pv[:, :, 1, :],
                    )
                    prev = nxt
                nc.vector.tensor_reduce(
                    drcol[:, 4 * mq : 4 * mq + 4],
                    prev.rearrange("p (a c) -> p a c", a=4),
                    axis=X, op=AL.max,
                )

            # ---- finals ----
            # dl partition-axis max via PE transposes (bf16) + DVE reduces;
            # the big GpSimd C-axis reduce measured ~160us on HW - unusable.
            dlm = fin_pool.tile([128, N], mm_dt, tag="dlm")
            nc.vector.tensor_max(dlm, run2[:, 0:N], run2[:, N : 2 * N])
            dlvals = fin_pool.tile([128, MT], f32, tag="dlvals")
            for th in range(2):
                tp = tp_psum.tile([128, 1024], mm_dt, tag="tp")
                for c in range(8):
                    blk = 8 * th + c
                    nc.tensor.transpose(
                        tp[:, c * 128 : (c + 1) * 128],
                        dlm[:, blk * 128 : (blk + 1) * 128],
                        identity,
                    )
                nc.vector.tensor_reduce(
                    dlvals[:, 8 * th : 8 * th + 8],
                    tp.rearrange("p (c j) -> p c j", c=8),
                    axis=X, op=AL.max,
                )
            # per-partition sums of dr and dl, then one tiny GpSimd C-add
            drs = fin_pool.tile([128, 1], f32, tag="drs")
            nc.vector.tensor_reduce(drs, drcol, axis=X, op=AL.add)
            dls = fin_pool.tile([128, 1], f32, tag="dls")
            nc.vector.tensor_reduce(dls, dlvals, axis=X, op=AL.add)
            tot = fin_pool.tile([128, 1], f32, tag="tot")
            nc.vector.tensor_add(tot, drs, dls)
            totsum = fin_pool.tile([1, 1], f32, tag="totsum")
            nc.gpsimd.tensor_reduce(totsum, tot, axis=C, op=AL.add)
            nc.vector.tensor_scalar_mul(
                out_sb[0:1, b : b + 1], totsum, -2.0 / N
            )

        nc.sync.dma_start(out=o[0:1, 0:BPC], in_=out_sb)

    nc.compile()
    return nc


def _get_nc(matmul_dtype="bfloat16"):
    key = matmul_dtype
    if key not in _CACHE:
        _CACHE[key] = _build(matmul_dtype)
    return _CACHE[key]


def kernel(x: np.ndarray, y: np.ndarray) -> np.ndarray:
    from concourse.bass_utils import run_bass_kernel_spmd

    x = np.ascontiguousarray(np.asarray(x, dtype=np.float32))
    y = np.ascontiguousarray(np.asarray(y, dtype=np.float32))
    nc = _get_nc()
    in_maps = [
        {"x": x[c * BPC : (c + 1) * BPC], "y": y[c * BPC : (c + 1) * BPC]}
        for c in range(NCORES)
    ]
    res = run_bass_kernel_spmd(nc, in_maps, core_ids=list(range(NCORES)))
    return np.concatenate([r["o"].reshape(BPC) for r in res.results])


# revision 24
# speedup vs baseline: 3.8481x; 1.0048x over previous
"""Chamfer distance loss kernel for Trainium2 (Bass/Tile), 8-core data parallel.

Problem: x, y [16, 2048, 3] fp32. Per batch b:
    P[i,j] = |x_i|^2 + |y_j|^2 - 2 x_i.y_j
    loss[b] = mean_j min_i P[i,j] + mean_i min_j P[i,j]

Strategy:
  - Shard batch dim: 2 batches per core across 8 cores, no cross-core comm.
  - P = -2*Q with Q[i,j] = x_i.y_j - 0.5|x_i|^2 - 0.5|y_j|^2, computed as ONE
    K=24 augmented matmul (bf16 triple-split for accuracy ~2^-27).
    min P == -2 * max Q.
  - Per m-tile the [128,2048] PSUM block is extracted once by an ACT copy
    to bf16 SBUF; dr (row max) runs as pair-batched bf16 max-trees on DVE
    (2x perf mode); dl (col max across the 16 m-tiles) runs as one paired
    [128,2N] running-max chain on DVE, merged at the end; the partition-axis
    max/add use GpSimd C-axis reduces (no PE transposes).
  - Means via ACT sum-accumulator (dl, with -2/N folded into the scale) and
    a DVE row-sum + GpSimd C-axis add (dr).
"""

import sys

if "/opt/trn_rl_repo" not in sys.path:
    sys.path.insert(0, "/opt/trn_rl_repo")

import numpy as np

B, N, D = 16, 2048, 3
NCORES = 8
BPC = B // NCORES  # batches per core
MT = N // 128  # 16 m-tiles

_CACHE = {}


def _build(matmul_dtype="float16"):
    from contextlib import ExitStack

    import concourse.bass as bass
    import concourse.mybir as mybir
    import concourse.tile as tile
    from concourse import bacc
    from concourse.masks import make_identity

    f32 = mybir.dt.float32
    mm_dt = getattr(mybir.dt, matmul_dtype)
    AL = mybir.AluOpType
    X = mybir.AxisListType.X
    C = mybir.AxisListType.C

    nc = bacc.Bacc()
    x = nc.dram_tensor("x", [BPC, N, D], f32, kind="ExternalInput")
    y = nc.dram_tensor("y", [BPC, N, D], f32, kind="ExternalInput")
    o = nc.dram_tensor("o", [1, BPC], f32, kind="ExternalOutput")
    # DRAM bounce buffer for the aug assembly (SBUF->SBUF transposes can't
    # be expressed as one DMA; DRAM APs have no partition-dim constraint)
    scratch = nc.dram_tensor("augscratch", [BPC, 128, 2 * 13 * 16], mm_dt,
                             kind="Internal")

    with tile.TileContext(nc) as tc, ExitStack() as ctx:
        singles = ctx.enter_context(tc.tile_pool(name="singles", bufs=1))
        aug_pool = ctx.enter_context(tc.tile_pool(name="aug", bufs=2))
        nat_pool = ctx.enter_context(tc.tile_pool(name="nat", bufs=2))
        small_pool = ctx.enter_context(tc.tile_pool(name="small", bufs=3))
        cp_pool = ctx.enter_context(tc.tile_pool(name="cp", bufs=4))
        tree_pool = ctx.enter_context(tc.tile_pool(name="tree", bufs=2))
        run_pool = ctx.enter_context(tc.tile_pool(name="run", bufs=2))
        fin_pool = ctx.enter_context(tc.tile_pool(name="fin", bufs=2))
        mm_psum = ctx.enter_context(tc.tile_pool(name="mmps", bufs=3, space="PSUM"))
        tp_psum = ctx.enter_context(tc.tile_pool(name="tpps", bufs=2, space="PSUM"))

        out_sb = singles.tile([1, BPC], f32)
        identity0 = singles.tile([128, 128], f32)
        make_identity(nc, identity0)
        identity = singles.tile([128, 128], mm_dt)
        nc.vector.tensor_copy(identity, identity0)

        # fp16 double-split augmented matmul, K=13 rows per operand:
        #   x ~ xh+xl (fp16 levels ~1, 2^-11); kept products hh,hl,lh give
        #   x.y to ~2^-21 (fp16 products are exact in fp32 accumulate).
        #   Norms -0.5|x|^2 are 2-way split and paired with ones rows.
        # stage fields (unique, [128,16] each):
        #   2d, 2d+1 = h/l of component d; 6,7 = norm h/l; 8 = ones
        LROWS = []
        RROWS = []
        for d in range(D):
            h, l = 2 * d, 2 * d + 1
            LROWS += [h, h, l]
            RROWS += [h, l, h]
        LROWS += [6, 7, 8, 8]
        RROWS += [8, 8, 6, 7]
        K = len(LROWS)  # 13

        # persistent per-batch lstage buffers; ones slots memset once at t=0
        lstages_all = []
        for bi in range(BPC):
            ls = singles.tile([128, 2 * K * 16], mm_dt, name=f"lst{bi}")
            lstages_all.append(ls)
            for g2, rows2 in enumerate((LROWS, RROWS)):
                oslots = [r for r, f in enumerate(rows2) if f == 8]
                o0 = oslots[0]
                assert oslots == list(range(o0, o0 + len(oslots)))
                nc.vector.memset(
                    ls[:, (g2 * K + o0) * 16 : (g2 * K + o0 + len(oslots)) * 16],
                    1.0,
                )

        # slot layout per side: slot r of lstage holds the field ROWS[r], so
        # the whole aug block moves as ONE transposing DMA per side instead
        # of 24 per-row DMAs (whose fixed DGE cost dominated the kernel).
        def slot_plan(rows):
            first = {}
            dups = []  # (src_slot, dst_slot)
            for r, f in enumerate(rows):
                if f == 8:
                    continue  # ones slots are memset directly
                if f in first:
                    dups.append((first[f], r))
                else:
                    first[f] = r
            return first, dups

        for b in range(BPC):
            Q = N // 128  # points per partition
            # both sides' slot data in one tile: slot gi*K+r = row r of side gi
            lstage = lstages_all[b]
            aug = aug_pool.tile([K, 2 * N], mm_dt, tag="aug")
            stv = lstage.rearrange("p (s q) -> p s q", s=2 * K)
            for gi, (side, src, rows) in enumerate(
                (("x", x, LROWS), ("y", y, RROWS))
            ):
                first, dups = slot_plan(rows)
                go = gi * K
                dma_eng = nc.sync if gi == 0 else nc.scalar
                nat = nat_pool.tile([128, Q * D], f32, tag=f"nat{side}")
                dma_eng.dma_start(
                    out=nat, in_=src[b].rearrange("(p q) d -> p (q d)", p=128)
                )
                sq = nat_pool.tile([128, Q * D], f32, tag=f"sq{side}")
                nc.vector.tensor_mul(sq, nat, nat)
                nrm = small_pool.tile([128, Q], f32, tag=f"nrm{side}")
                nc.vector.tensor_reduce(
                    nrm, sq.rearrange("p (q d) -> p q d", d=D), axis=X,
                    op=AL.add,
                )
                nc.vector.tensor_scalar_mul(nrm, nrm, -0.5)

                def split2(val_f32, fidx, pool_tag):
                    # val_f32: [128, Q] fp32; writes fp16 h/l into
                    # the two lstage slots in fidx
                    t1 = nat_pool.tile([128, Q], f32, tag=f"{pool_tag}t1")
                    nc.vector.tensor_copy(fidx[0], val_f32)          # h
                    nc.vector.tensor_sub(t1, val_f32, fidx[0])
                    nc.vector.tensor_copy(fidx[1], t1)               # l

                natv = nat.rearrange("p (q d) -> p d q", d=D)
                for d in range(D):
                    split2(
                        natv[:, d, :],
                        [stv[:, go + first[2 * d + j], :] for j in range(2)],
                        f"c{side}",
                    )
                split2(
                    nrm, [stv[:, go + first[6 + j], :] for j in range(2)],
                    f"n{side}",
                )
                # dup slots: batch 0 on DVE (in-engine, startup latency);
                # batch 1 on GpSimd (fully overlapped)
                dup_eng = nc.vector if b == 0 else nc.gpsimd
                for src_slot, dst_slot in dups:
                    dup_eng.tensor_copy(
                        stv[:, go + dst_slot, :], stv[:, go + src_slot, :]
                    )
                # aug assembly via DRAM bounce, per side on its own queue:
                #   aug[r, g*N + p*16+q] = lstage[p, (g*K+r)*16+q]
                g = gi
                dma_eng.dma_start(
                    out=scratch[b][:, g * K * Q : (g + 1) * K * Q],
                    in_=lstage[:, g * K * Q : (g + 1) * K * Q],
                )
                dma_eng.dma_start(
                    out=aug[0:K, g * N : (g + 1) * N].rearrange(
                        "r (p q) -> r p q", p=128
                    ),
                    in_=scratch[b][:, g * K * Q : (g + 1) * K * Q].rearrange(
                        "p (r q) -> r p q", r=K
                    ),
                )

            lhsT = aug[:, 0:N]
            rhs = aug[:, N : 2 * N]

            # ---- main loop, m-tile PAIRS ----
            # Extraction is all-ACT (copy PSUM -> bf16 SBUF). dr runs as a
            # pair-batched bf16 max-tree on DVE (2x mode). dl runs as one
            # paired running-max chain [128, 2N] holding even tiles in the
            # left half, odd in the right, merged at the end.
            drcol = fin_pool.tile([128, MT], f32, tag="drcol")
            run2 = run_pool.tile([128, 2 * N], mm_dt, tag="run2")
            for mq in range(MT // 4):
                # two pair-tiles per quad so DVE work starts after 4 ACT
                # copies instead of 8
                t1 = tree_pool.tile([128, 4096], mm_dt, tag="t1024")
                for ph in range(2):
                    cpx = cp_pool.tile([128, 2 * N], mm_dt, tag="cp")
                    for quar in range(2):
                        m = 4 * mq + 2 * ph + quar
                        for jh in range(2):
                            psg = mm_psum.tile([128, N // 2], f32, tag="mm")
                            for n in range(2):
                                nc.tensor.matmul(
                                    psg[:, n * 512 : (n + 1) * 512],
                                    lhsT=lhsT[:, m * 128 : (m + 1) * 128],
                                    rhs=rhs[
                                        :,
                                        (2 * jh + n) * 512 : (2 * jh + n + 1)
                                        * 512,
                                    ],
                                    start=True,
                                    stop=True,
                                )
                            nc.scalar.copy(
                                cpx[
                                    :,
                                    quar * N + jh * (N // 2) : quar * N
                                    + (jh + 1) * (N // 2),
                                ],
                                psg,
                            )
                    # dl chain update (serial chain -> first priority)
                    if mq == 0 and ph == 0:
                        runinit = cpx  # run2 = max(pair0, pair1) below
                    elif mq == 0 and ph == 1:
                        nc.vector.tensor_max(run2, runinit, cpx)
                    else:
                        nc.vector.tensor_max(run2, run2, cpx)
                    # dr tree level 0 for this pair
                    pv = cpx.rearrange("p (a h j) -> p a h j", a=2, h=2)
                    nc.vector.tensor_max(
                        t1.rearrange("p (b a j) -> p b a j", b=2, a=2)[:, ph],
                        pv[:, :, 0, :], pv[:, :, 1, :],
                    )
                # dr quad-tree levels 1+: 4 tiles batched
                widths = [512, 256, 128]
                prev = t1
                for w in widths:
                    nxt = tree_pool.tile([128, 4 * w], mm_dt, tag=f"t{w}")
                    pv = prev.rearrange("p (a h j) -> p a h j", a=4, h=2)
                    nc.vector.tensor_max(
                        nxt.rearrange("p (a j) -> p a j", a=4),
                        pv[:, :, 0, :], pv[:, :, 1, :],
                    )
                    prev = nxt
                nc.vector.tensor_reduce(
                    drcol[:, 4 * mq : 4 * mq + 4],
                    prev.rearrange("p (a c) -> p a c", a=4),
                    axis=X, op=AL.max,
                )

            # ---- finals ----
            # dl partition-axis max via PE transposes (bf16) + DVE reduces;
            # the big GpSimd C-axis reduce measured ~160us on HW - unusable.
            dlm = fin_pool.tile([128, N], mm_dt, tag="dlm")
            nc.vector.tensor_max(dlm, run2[:, 0:N], run2[:, N : 2 * N])
            dlvals = fin_pool.tile([128, MT], f32, tag="dlvals")
            for th in range(2):
                tp = tp_psum.tile([128, 1024], mm_dt, tag="tp")
                for c in range(8):
                    blk = 8 * th + c
                    nc.tensor.transpose(
                        tp[:, c * 128 : (c + 1) * 128],
                        dlm[:, blk * 128 : (blk + 1) * 128],
                        identity,
                    )
                nc.vector.tensor_reduce(
                    dlvals[:, 8 * th : 8 * th + 8],
                    tp.rearrange("p (c j) -> p c j", c=8),
                    axis=X, op=AL.max,
                )
            # per-partition sums of dr and dl, then one tiny GpSimd C-add
            drs = fin_pool.tile([128, 1], f32, tag="drs")
            nc.vector.tensor_reduce(drs, drcol, axis=X, op=AL.add)
            dls = fin_pool.tile([128, 1], f32, tag="dls")
            nc.vector.tensor_reduce(dls, dlvals, axis=X, op=AL.add)
            tot = fin_pool.tile([128, 1], f32, tag="tot")
            nc.vector.tensor_add(tot, drs, dls)
            totsum = fin_pool.tile([1, 1], f32, tag="totsum")
            nc.gpsimd.tensor_reduce(totsum, tot, axis=C, op=AL.add)
            nc.vector.tensor_scalar_mul(
                out_sb[0:1, b : b + 1], totsum, -2.0 / N
            )

        nc.sync.dma_start(out=o[0:1, 0:BPC], in_=out_sb)

    nc.compile()
    return nc


def _get_nc(matmul_dtype="bfloat16"):
    key = matmul_dtype
    if key not in _CACHE:
        _CACHE[key] = _build(matmul_dtype)
    return _CACHE[key]


def kernel(x: np.ndarray, y: np.ndarray) -> np.ndarray:
    from concourse.bass_utils import run_bass_kernel_spmd

    x = np.ascontiguousarray(np.asarray(x, dtype=np.float32))
    y = np.ascontiguousarray(np.asarray(y, dtype=np.float32))
    nc = _get_nc()
    in_maps = [
        {"x": x[c * BPC : (c + 1) * BPC], "y": y[c * BPC : (c + 1) * BPC]}
        for c in range(NCORES)
    ]
    res = run_bass_kernel_spmd(nc, in_maps, core_ids=list(range(NCORES)))
    return np.concatenate([r["o"].reshape(BPC) for r in res.results])
